# revision 4
# baseline (speedup 1.0000x reference)
"""ConMGIN (dual-graph GIN + spatial attention + decoder heads) on 8 Trainium2
NeuronCores via Bass/Tile.

Strategy (graph/data parallel, per sharding hint):
  - Nodes sharded across 8 cores (6250 rows each, padded to 6272 = 49*128).
  - y = x @ Wg1a computed row-parallel, AllGathered to a replicated y table
    (the GIN-conv MLP commutes with the linear aggregation, so features are
    projected 512->256 BEFORE the per-edge gather).
  - Edges bucketed by dst core + dst chunk (128 rows) on the host; per-edge
    source rows fetched with indirect DMA gathers (128 rows/instr); segment
    sum realized as one-hot scatter matmuls accumulating in PSUM per chunk.
  - Layer 2 aggregates z = h @ Wg2a (64 wide) the same way after a second
    AllGather.
  - Attention/MLP/decoder/heads are row-parallel with replicated weights.

Self-contained: hardcodes the problem shapes; only needs numpy + concourse
(from /opt/trn_rl_repo) + 8 axon trn2 cores.
"""
import sys

for _p in ('/opt/trn_rl_repo',):
    if _p not in sys.path:
        sys.path.insert(0, _p)

import numpy as np

import concourse.bass as bass
import concourse.bacc as bacc
import concourse.tile as tile
from concourse import mybir, bass_utils

F32 = mybir.dt.float32
I32 = mybir.dt.int32
AF = mybir.ActivationFunctionType
OP = mybir.AluOpType

NC = 8          # cores
P = 128         # partitions
BN_EPS = 1e-5


# ---------------------------------------------------------------- host prep
def _edge_plan(src, dst, g, nper, PN, nchunk, zoff_stride):
    """Per-core padded tile streams for one graph.

    Returns (T, yidx, zidx, dstv):
      T: [nchunk] tiles per chunk (max over cores, shared loop bounds)
      yidx: [NC, 128, Ttot] int32  y-table row per edge slot
      zidx: [NC, 128, Ttot] int32  z-table row per edge slot
      dstv: [NC, 128, Ttot] f32    dst slot within chunk (-1 for padding)
    """
    core = dst // nper
    loc = dst - core * nper
    chunk = loc // P
    slot = loc - chunk * P
    ysrc = (src // nper) * PN + (src % nper)
    zsrc = ((src // nper) * 2 + g) * PN + (src % nper)

    gid = (core * nchunk + chunk).astype(np.int64)
    order = np.argsort(gid, kind='stable')
    gid_s = gid[order]
    counts = np.bincount(gid_s, minlength=NC * nchunk)
    starts = np.concatenate([[0], np.cumsum(counts)[:-1]])
    rank = np.arange(len(gid_s)) - starts[gid_s]

    cnt2 = counts.reshape(NC, nchunk)
    T = (cnt2.max(axis=0) + P - 1) // P          # tiles per chunk
    T = np.maximum(T, 1)
    Ttot = int(T.sum())
    tbase = np.concatenate([[0], np.cumsum(T)[:-1]])  # tile base per chunk

    yidx = np.zeros((NC, Ttot * P), np.int32)
    zidx = np.zeros((NC, Ttot * P), np.int32)
    dstv = np.full((NC, Ttot * P), -1.0, np.float32)

    pos = tbase[chunk[order]] * P + rank
    c_s = core[order]
    yidx[c_s, pos] = ysrc[order]
    zidx[c_s, pos] = zsrc[order]
    dstv[c_s, pos] = slot[order].astype(np.float32)

    # [NC, Ttot*P] -> [NC, 128, Ttot] (partition p = edge slot p of tile t)
    yidx = yidx.reshape(NC, Ttot, P).transpose(0, 2, 1).copy()
    zidx = zidx.reshape(NC, Ttot, P).transpose(0, 2, 1).copy()
    dstv = dstv.reshape(NC, Ttot, P).transpose(0, 2, 1).copy()
    return T.astype(int), yidx, zidx, dstv


def make_plan(N, NFEAT, NHID1, NHID2, ATT_H, inputs):
    nper = N // NC
    assert nper * NC == N
    PN = ((nper + P - 1) // P) * P
    nchunk = PN // P

    plan = dict(N=N, nper=nper, PN=PN, nchunk=nchunk,
                NFEAT=NFEAT, NHID1=NHID1, NHID2=NHID2, ATT_H=ATT_H)

    f32 = lambda a: np.ascontiguousarray(np.asarray(a), dtype=np.float32)

    # per-graph edge plans
    Ts, yidxs, zidxs, dstvs = [], [], [], []
    for g, key in enumerate(('sadj_edges', 'fadj_edges')):
        e = np.asarray(inputs[key])
        T, yi, zi, dv = _edge_plan(e[0].astype(np.int64), e[1].astype(np.int64),
                                   g, nper, PN, nchunk, None)
        Ts.append(T); yidxs.append(yi); zidxs.append(zi); dstvs.append(dv)
    plan['T'] = Ts

    # x shards, zero padded
    x = f32(inputs['x'])
    x_pad = np.zeros((NC, PN, NFEAT), np.float32)
    x_pad[:, :nper, :] = x.reshape(NC, nper, NFEAT)

    # weights, host-rearranged to SBUF layouts
    def wsplit(w, kchunks, ncols):
        # [K, ncols] -> [128, kchunks*ncols] (chunk-major columns)
        out = np.zeros((P, kchunks, ncols), np.float32)
        for c in range(kchunks):
            rows = w[c * P:(c + 1) * P]
            out[:rows.shape[0], c, :] = rows
        return out.reshape(P, kchunks * ncols)

    W1a = f32(inputs['Wg1a']); W1b = f32(inputs['Wg1b'])
    W2a = f32(inputs['Wg2a']); W2b = f32(inputs['Wg2b'])
    Wa1 = f32(inputs['Wa1']); Wa2 = f32(inputs['Wa2'])
    Wmlp = f32(inputs['Wmlp']); Wd = f32(inputs['Wd'])
    Wpi = f32(inputs['Wpi']); Wv = f32(inputs['Wv']); Wmu = f32(inputs['Wmu'])

    bn_scale = f32(inputs['bn_gamma']) / np.sqrt(f32(inputs['bn_var']) + BN_EPS)
    Wd_f = Wd * bn_scale[None, :]
    bd_f = (f32(inputs['bd']) - f32(inputs['bn_mean'])) * bn_scale + f32(inputs['bn_beta'])

    bc = lambda v: np.broadcast_to(f32(v)[None, :], (P, len(v))).copy()

    const = dict(
        W1a_r=wsplit(W1a, 4, NHID1),                    # [128,4,256]
        W1b_r=wsplit(W1b, 2, NHID1),                    # [128,2,256]
        W2a_r=wsplit(W2a, 2, NHID2),                    # [128,2,64]
        W2b_r=np.pad(W2b, ((0, P - NHID2), (0, 0))),    # [128,64]
        Wa1_r=np.pad(Wa1, ((0, P - NHID2), (0, 0))),    # [128,16]
        Wmlp_r=np.pad(Wmlp, ((0, P - NHID2), (0, 0))),  # [128,64]
        Wd_r=np.pad(Wd_f, ((0, P - NHID2), (0, 0))),    # [128,256]
        Wpi_r=wsplit(Wpi, 2, NFEAT),                    # [128,2,512]
        Wv_r=wsplit(Wv, 2, NFEAT),
        Wmu_r=wsplit(Wmu, 2, NFEAT),
        b1a_b=bc(inputs['bg1a']),                       # [128,256]
        b1b_pp=f32(inputs['bg1b']).reshape(2, P).T.copy(),  # [128,2]
        b2a_b=bc(inputs['bg2a']),                       # [128,64]
        b2b_b=bc(inputs['bg2b']),                       # [128,64]
        ba1_b=bc(inputs['ba1']),                        # [128,16]
        Wa2_b=np.broadcast_to(Wa2[:, 0][None, :], (P, ATT_H)).copy(),
        bmlp_b=bc(inputs['bmlp']),                      # [128,64]
        bdec_b=bc(bd_f),                                # [128,256]
        bpi_b=bc(inputs['bpi']), bv_b=bc(inputs['bv']), bmu_b=bc(inputs['bmu']),
        ident=np.eye(P, dtype=np.float32),
        iota=np.broadcast_to(np.arange(P, dtype=np.float32)[None, :], (P, P)).copy(),
    )

    in_maps = []
    for c in range(NC):
        m = {'x_pad': x_pad[c]}
        m.update({k: v for k, v in const.items()})
        for g in range(2):
            m[f'yidx{g}'] = yidxs[g][c]
            m[f'zidx{g}'] = zidxs[g][c]
            m[f'dstv{g}'] = dstvs[g][c]
        in_maps.append(m)
    return plan, in_maps


# ------------------------------------------------------------- bass program
def build_program(plan):
    PN, nchunk = plan['PN'], plan['nchunk']
    NFEAT, NHID1, NHID2, ATT_H = (plan['NFEAT'], plan['NHID1'],
                                  plan['NHID2'], plan['ATT_H'])
    T = plan['T']               # [2][nchunk]
    Ttot = [int(sum(t)) for t in T]
    tbase = [np.concatenate([[0], np.cumsum(t)[:-1]]).astype(int) for t in T]
    KF = NFEAT // P             # 4
    KH = NHID1 // P             # 2

    nc = bacc.Bacc('TRN2', target_bir_lowering=False, debug=False,
                   num_devices=NC)

    # ---- I/O
    x_pad = nc.dram_tensor('x_pad', [PN, NFEAT], F32, kind='ExternalInput')
    ins = {}
    for name, shape in [
        ('W1a_r', [P, KF * NHID1]), ('W1b_r', [P, KH * NHID1]),
        ('W2a_r', [P, KH * NHID2]), ('W2b_r', [P, NHID2]),
        ('Wa1_r', [P, ATT_H]), ('Wmlp_r', [P, NHID2]), ('Wd_r', [P, NHID1]),
        ('Wpi_r', [P, KH * NFEAT]), ('Wv_r', [P, KH * NFEAT]),
        ('Wmu_r', [P, KH * NFEAT]),
        ('b1a_b', [P, NHID1]), ('b1b_pp', [P, KH]), ('b2a_b', [P, NHID2]),
        ('b2b_b', [P, NHID2]), ('ba1_b', [P, ATT_H]), ('Wa2_b', [P, ATT_H]),
        ('bmlp_b', [P, NHID2]), ('bdec_b', [P, NHID1]),
        ('bpi_b', [P, NFEAT]), ('bv_b', [P, NFEAT]), ('bmu_b', [P, NFEAT]),
        ('ident', [P, P]), ('iota', [P, P]),
    ]:
        ins[name] = nc.dram_tensor(name, shape, F32, kind='ExternalInput')
    for g in range(2):
        ins[f'yidx{g}'] = nc.dram_tensor(f'yidx{g}', [P, Ttot[g]], I32, kind='ExternalInput')
        ins[f'zidx{g}'] = nc.dram_tensor(f'zidx{g}', [P, Ttot[g]], I32, kind='ExternalInput')
        ins[f'dstv{g}'] = nc.dram_tensor(f'dstv{g}', [P, Ttot[g]], F32, kind='ExternalInput')

    outs = {}
    for name, w in [('emb1', NHID2), ('emb2', NHID2), ('emb', NHID2),
                    ('pi', NFEAT), ('var', NFEAT), ('mean', NFEAT)]:
        outs[name] = nc.dram_tensor(name, [PN, w], F32, kind='ExternalOutput')

    # ---- internal DRAM
    y_own = nc.dram_tensor('y_own', [PN, NHID1], F32)
    y_ownb = nc.dram_tensor('y_ownb', [PN, NHID1], F32)
    y_table = nc.dram_tensor('y_table', [NC * PN, NHID1], F32, addr_space='Shared')
    z_own = nc.dram_tensor('z_own', [2 * PN, NHID2], F32)
    z_ownb = nc.dram_tensor('z_ownb', [2 * PN, NHID2], F32)
    z_table = nc.dram_tensor('z_table', [NC * 2 * PN, NHID2], F32, addr_space='Shared')

    with tile.TileContext(nc) as tc:
        with tc.tile_pool(name='const', bufs=1) as cp, \
             tc.tile_pool(name='stream', bufs=1) as strp, \
             tc.tile_pool(name='gath', bufs=16) as gp, \
             tc.tile_pool(name='sel', bufs=16) as sp, \
             tc.tile_pool(name='work', bufs=3) as wp, \
             tc.tile_pool(name='psum', bufs=2, space='PSUM') as pp:

            # ---- load constants into SBUF
            sb = {}
            for name, t in ins.items():
                if name.startswith(('yidx', 'zidx', 'dstv')):
                    continue
                shape = list(t.shape)
                tl = cp.tile(shape, F32, tag=name)
                nc.sync.dma_start(out=tl[:], in_=t[:])
                sb[name] = tl

            def psum(tag, shape):
                return pp.tile(shape, F32, tag=tag, name='ps_' + tag)

            def transpose_to(dst_ap, src_ap):
                pt = psum('small', [P, P])
                nc.tensor.transpose(out=pt[:src_ap.shape[1], :src_ap.shape[0]],
                                    in_=src_ap, identity=sb['ident'][:])
                nc.vector.tensor_copy(out=dst_ap,
                                      in_=pt[:dst_ap.shape[0], :dst_ap.shape[1]])

            # ================= P0: y = x @ W1a (row-parallel) ==============
            for k in range(nchunk):
                xt = wp.tile([P, NFEAT], F32, tag='x')
                nc.sync.dma_start(out=xt[:], in_=x_pad[k * P:(k + 1) * P, :])
                xT = wp.tile([P, NFEAT], F32, tag='xT')
                for ci in range(KF):
                    transpose_to(xT[:, ci * P:(ci + 1) * P],
                                 xt[:, ci * P:(ci + 1) * P])
                py = psum('agg', [P, NHID1])
                for ci in range(KF):
                    nc.tensor.matmul(py[:], lhsT=xT[:, ci * P:(ci + 1) * P],
                                     rhs=sb['W1a_r'][:, bass.ts(ci, NHID1)],
                                     start=(ci == 0), stop=(ci == KF - 1))
                ys = wp.tile([P, NHID1], F32, tag='ys')
                nc.vector.tensor_copy(out=ys[:], in_=py[:])
                nc.sync.dma_start(out=y_own[k * P:(k + 1) * P, :], in_=ys[:])
                yb = wp.tile([P, NHID1], F32, tag='yb')
                nc.vector.tensor_add(out=yb[:], in0=py[:], in1=sb['b1a_b'][:])
                nc.sync.dma_start(out=y_ownb[k * P:(k + 1) * P, :], in_=yb[:])

            nc.gpsimd.collective_compute(
                'AllGather', OP.bypass,
                replica_groups=[list(range(NC))],
                ins=[y_own.ap().opt()], outs=[y_table.ap().opt()])

            # ================= P1: GIN layer 1 per graph ===================
            def build_S(t, dstv_sb):
                S = sp.tile([P, P], F32, tag='S', name='S')
                nc.vector.tensor_tensor(
                    out=S[:], in0=dstv_sb[:, t:t + 1].to_broadcast([P, P]),
                    in1=sb['iota'][:], op=OP.is_equal)
                return S

            for g in range(2):
                yidx_sb = strp.tile([P, Ttot[g]], I32, tag='yidx')
                nc.sync.dma_start(out=yidx_sb[:], in_=ins[f'yidx{g}'][:, :])
                dstv_sb = strp.tile([P, Ttot[g]], F32, tag='dstv')
                nc.sync.dma_start(out=dstv_sb[:], in_=ins[f'dstv{g}'][:, :])

                for k in range(nchunk):
                    nt = int(T[g][k])
                    pagg = psum('agg', [P, NHID1])
                    for i in range(nt):
                        t = int(tbase[g][k]) + i
                        G = gp.tile([P, NHID1], F32, tag='G1')
                        nc.gpsimd.indirect_dma_start(
                            out=G[:], out_offset=None, in_=y_table[:, :],
                            in_offset=bass.IndirectOffsetOnAxis(
                                ap=yidx_sb[:, t:t + 1], axis=0))
                        S = build_S(t, dstv_sb)
                        nc.tensor.matmul(pagg[:], lhsT=S[:], rhs=G[:],
                                         start=(i == 0), stop=(i == nt - 1))
                    # epilogue: h_pre = relu(agg + y_own + b1a)
                    yb = wp.tile([P, NHID1], F32, tag='yb2')
                    nc.sync.dma_start(out=yb[:], in_=y_ownb[k * P:(k + 1) * P, :])
                    hp = wp.tile([P, NHID1], F32, tag='hp')
                    nc.vector.tensor_add(out=hp[:], in0=pagg[:], in1=yb[:])
                    hpr = wp.tile([P, NHID1], F32, tag='hpr')
                    nc.scalar.activation(out=hpr[:], in_=hp[:], func=AF.Relu)
                    hpT = wp.tile([P, NHID1], F32, tag='hpT')
                    for ci in range(KH):
                        transpose_to(hpT[:, ci * P:(ci + 1) * P],
                                     hpr[:, ci * P:(ci + 1) * P])
                    hT = wp.tile([P, NHID1], F32, tag='hT')
                    for fo in range(KH):
                        ph = psum('small', [P, P])
                        for ci in range(KH):
                            nc.tensor.matmul(
                                ph[:], lhsT=sb['W1b_r'][:, ci * NHID1 + fo * P:ci * NHID1 + (fo + 1) * P],
                                rhs=hpT[:, ci * P:(ci + 1) * P],
                                start=(ci == 0), stop=(ci == KH - 1))
                        nc.scalar.activation(out=hT[:, fo * P:(fo + 1) * P],
                                             in_=ph[:], func=AF.Relu,
                                             bias=sb['b1b_pp'][:, fo:fo + 1])
                    pz = psum('z', [P, NHID2])
                    for ci in range(KH):
                        nc.tensor.matmul(pz[:], lhsT=hT[:, ci * P:(ci + 1) * P],
                                         rhs=sb['W2a_r'][:, bass.ts(ci, NHID2)],
                                         start=(ci == 0), stop=(ci == KH - 1))
                    zt = wp.tile([P, NHID2], F32, tag='zt')
                    nc.vector.tensor_copy(out=zt[:], in_=pz[:])
                    nc.sync.dma_start(
                        out=z_own[g * PN + k * P:g * PN + (k + 1) * P, :], in_=zt[:])
                    zb = wp.tile([P, NHID2], F32, tag='zb')
                    nc.vector.tensor_add(out=zb[:], in0=pz[:], in1=sb['b2a_b'][:])
                    nc.sync.dma_start(
                        out=z_ownb[g * PN + k * P:g * PN + (k + 1) * P, :], in_=zb[:])

            nc.gpsimd.collective_compute(
                'AllGather', OP.bypass,
                replica_groups=[list(range(NC))],
                ins=[z_own.ap().opt()], outs=[z_table.ap().opt()])

            # ================= P2: GIN layer 2 per graph ===================
            for g in range(2):
                zidx_sb = strp.tile([P, Ttot[g]], I32, tag='yidx')
                nc.sync.dma_start(out=zidx_sb[:], in_=ins[f'zidx{g}'][:, :])
                dstv_sb = strp.tile([P, Ttot[g]], F32, tag='dstv')
                nc.sync.dma_start(out=dstv_sb[:], in_=ins[f'dstv{g}'][:, :])
                embout = outs['emb1'] if g == 0 else outs['emb2']

                for k in range(nchunk):
                    nt = int(T[g][k])
                    paggz = psum('z', [P, NHID2])
                    for i in range(nt):
                        t = int(tbase[g][k]) + i
                        G2 = gp.tile([P, NHID2], F32, tag='G2')
                        nc.gpsimd.indirect_dma_start(
                            out=G2[:], out_offset=None, in_=z_table[:, :],
                            in_offset=bass.IndirectOffsetOnAxis(
                                ap=zidx_sb[:, t:t + 1], axis=0))
                        S = build_S(t, dstv_sb)
                        nc.tensor.matmul(paggz[:], lhsT=S[:], rhs=G2[:],
                                         start=(i == 0), stop=(i == nt - 1))
                    zb = wp.tile([P, NHID2], F32, tag='zb2')
                    nc.sync.dma_start(out=zb[:],
                                      in_=z_ownb[g * PN + k * P:g * PN + (k + 1) * P, :])
                    p2 = wp.tile([P, NHID2], F32, tag='p2')
                    nc.vector.tensor_add(out=p2[:], in0=paggz[:], in1=zb[:])
                    p2r = wp.tile([P, NHID2], F32, tag='p2r')
                    nc.scalar.activation(out=p2r[:], in_=p2[:], func=AF.Relu)
                    p2T = wp.tile([P, P], F32, tag='p2T')
                    transpose_to(p2T[:NHID2, :], p2r[:])
                    pe = psum('z2', [P, NHID2])
                    nc.tensor.matmul(pe[:], lhsT=p2T[:NHID2, :],
                                     rhs=sb['W2b_r'][:NHID2, :],
                                     start=True, stop=True)
                    eg = wp.tile([P, NHID2], F32, tag='eg')
                    nc.vector.tensor_add(out=eg[:], in0=pe[:], in1=sb['b2b_b'][:])
                    nc.sync.dma_start(out=embout[k * P:(k + 1) * P, :], in_=eg[:])

            # ================= P3: attention + decoder + heads =============
            for k in range(nchunk):
                st = wp.tile([P, NHID2], F32, tag='st')
                nc.sync.dma_start(out=st[:], in_=outs['emb1'][k * P:(k + 1) * P, :])
                ft = wp.tile([P, NHID2], F32, tag='ft')
                nc.sync.dma_start(out=ft[:], in_=outs['emb2'][k * P:(k + 1) * P, :])
                cm = wp.tile([P, NHID2], F32, tag='cm')
                nc.vector.tensor_add(out=cm[:], in0=st[:], in1=ft[:])
                nc.scalar.mul(out=cm[:], in_=cm[:], mul=0.5)

                wstk = wp.tile([P, 3], F32, tag='wstk')
                for vi, v in enumerate((st, cm, ft)):
                    vT = wp.tile([P, P], F32, tag='vT')
                    transpose_to(vT[:NHID2, :], v[:])
                    pa = psum('z2', [P, ATT_H])
                    nc.tensor.matmul(pa[:], lhsT=vT[:NHID2, :],
                                     rhs=sb['Wa1_r'][:NHID2, :],
                                     start=True, stop=True)
                    av = wp.tile([P, ATT_H], F32, tag='av')
                    nc.vector.tensor_add(out=av[:], in0=pa[:], in1=sb['ba1_b'][:])
                    at = wp.tile([P, ATT_H], F32, tag='at')
                    nc.scalar.activation(out=at[:], in_=av[:], func=AF.Tanh)
                    aw = wp.tile([P, ATT_H], F32, tag='aw')
                    nc.vector.tensor_tensor(out=aw[:], in0=at[:],
                                            in1=sb['Wa2_b'][:], op=OP.mult)
                    nc.vector.reduce_sum(out=wstk[:, vi:vi + 1], in_=aw[:],
                                         axis=mybir.AxisListType.X)
                mx = wp.tile([P, 1], F32, tag='mx')
                nc.vector.reduce_max(out=mx[:], in_=wstk[:], axis=mybir.AxisListType.X)
                ws = wp.tile([P, 3], F32, tag='ws')
                nc.vector.tensor_scalar(out=ws[:], in0=wstk[:], scalar1=mx[:],
                                        scalar2=None, op0=OP.subtract)
                we = wp.tile([P, 3], F32, tag='we')
                nc.scalar.activation(out=we[:], in_=ws[:], func=AF.Exp)
                sm = wp.tile([P, 1], F32, tag='sm')
                nc.vector.reduce_sum(out=sm[:], in_=we[:], axis=mybir.AxisListType.X)
                rc = wp.tile([P, 1], F32, tag='rc')
                nc.vector.reciprocal(out=rc[:], in_=sm[:])
                beta = wp.tile([P, 3], F32, tag='beta')
                nc.vector.tensor_scalar(out=beta[:], in0=we[:], scalar1=rc[:],
                                        scalar2=None, op0=OP.mult)

                ea = wp.tile([P, NHID2], F32, tag='ea')
                nc.vector.tensor_scalar(out=ea[:], in0=st[:],
                                        scalar1=beta[:, 0:1], scalar2=None,
                                        op0=OP.mult)
                tmp = wp.tile([P, NHID2], F32, tag='tmp')
                nc.vector.tensor_scalar(out=tmp[:], in0=cm[:],
                                        scalar1=beta[:, 1:2], scalar2=None,
                                        op0=OP.mult)
                nc.vector.tensor_add(out=ea[:], in0=ea[:], in1=tmp[:])
                nc.vector.tensor_scalar(out=tmp[:], in0=ft[:],
                                        scalar1=beta[:, 2:3], scalar2=None,
                                        op0=OP.mult)
                nc.vector.tensor_add(out=ea[:], in0=ea[:], in1=tmp[:])

                eaT = wp.tile([P, P], F32, tag='eaT')
                transpose_to(eaT[:NHID2, :], ea[:])
                pm = psum('z2', [P, NHID2])
                nc.tensor.matmul(pm[:], lhsT=eaT[:NHID2, :],
                                 rhs=sb['Wmlp_r'][:NHID2, :], start=True, stop=True)
                eo = wp.tile([P, NHID2], F32, tag='eo')
                nc.vector.tensor_add(out=eo[:], in0=pm[:], in1=sb['bmlp_b'][:])
                nc.sync.dma_start(out=outs['emb'][k * P:(k + 1) * P, :], in_=eo[:])

                eoT = wp.tile([P, P], F32, tag='eoT')
                transpose_to(eoT[:NHID2, :], eo[:])
                pd = psum('agg', [P, NHID1])
                nc.tensor.matmul(pd[:], lhsT=eoT[:NHID2, :],
                                 rhs=sb['Wd_r'][:NHID2, :], start=True, stop=True)
                hd = wp.tile([P, NHID1], F32, tag='hd')
                nc.vector.tensor_add(out=hd[:], in0=pd[:], in1=sb['bdec_b'][:])
                hdr = wp.tile([P, NHID1], F32, tag='hdr')
                nc.scalar.activation(out=hdr[:], in_=hd[:], func=AF.Relu)
                hdT = wp.tile([P, NHID1], F32, tag='hdT')
                for ci in range(KH):
                    transpose_to(hdT[:, ci * P:(ci + 1) * P],
                                 hdr[:, ci * P:(ci + 1) * P])

                for wname, bname, oname in [('Wpi_r', 'bpi_b', 'pi'),
                                            ('Wv_r', 'bv_b', 'var'),
                                            ('Wmu_r', 'bmu_b', 'mean')]:
                    phd = psum('agg', [P, NFEAT])
                    for ci in range(KH):
                        nc.tensor.matmul(phd[:], lhsT=hdT[:, ci * P:(ci + 1) * P],
                                         rhs=sb[wname][:, bass.ts(ci, NFEAT)],
                                         start=(ci == 0), stop=(ci == KH - 1))
                    hb = wp.tile([P, NFEAT], F32, tag='hb')
                    nc.vector.tensor_add(out=hb[:], in0=phd[:], in1=sb[bname][:])
                    ho = wp.tile([P, NFEAT], F32, tag='ho')
                    if oname == 'pi':
                        nc.scalar.activation(out=ho[:], in_=hb[:], func=AF.Sigmoid)
                    elif oname == 'var':
                        # softplus(x)=log(1+e^x) via Newton on e^t = 1+e^x
                        # (no log/softplus PWP table on this build)
                        up1 = wp.tile([P, NFEAT], F32, tag='sp_u')
                        nc.scalar.activation(out=up1[:], in_=hb[:], func=AF.Exp)
                        nc.vector.tensor_scalar(out=up1[:], in0=up1[:],
                                                scalar1=1.0, scalar2=None,
                                                op0=OP.add)
                        tcur = wp.tile([P, NFEAT], F32, tag='sp_t')
                        nc.scalar.activation(out=tcur[:], in_=hb[:], func=AF.Relu)
                        for _ in range(4):
                            wexp = wp.tile([P, NFEAT], F32, tag='sp_w')
                            nc.scalar.activation(out=wexp[:], in_=tcur[:],
                                                 func=AF.Exp, scale=-1.0)
                            nc.vector.tensor_tensor(out=wexp[:], in0=wexp[:],
                                                    in1=up1[:], op=OP.mult)
                            tnxt = wp.tile([P, NFEAT], F32, tag='sp_t2')
                            nc.vector.tensor_add(out=tnxt[:], in0=tcur[:],
                                                 in1=wexp[:])
                            tcur = wp.tile([P, NFEAT], F32, tag='sp_t')
                            nc.vector.tensor_scalar(out=tcur[:], in0=tnxt[:],
                                                    scalar1=-1.0, scalar2=None,
                                                    op0=OP.add)
                        nc.vector.tensor_scalar(out=ho[:], in0=tcur[:],
                                                scalar1=1e-4, scalar2=1e4,
                                                op0=OP.max, op1=OP.min)
                    else:
                        nc.scalar.activation(out=ho[:], in_=hb[:], func=AF.Exp)
                        nc.vector.tensor_scalar(out=ho[:], in0=ho[:],
                                                scalar1=1e-5, scalar2=1e6,
                                                op0=OP.max, op1=OP.min)
                    nc.sync.dma_start(out=outs[oname][k * P:(k + 1) * P, :],
                                      in_=ho[:])

    nc.compile()
    return nc


# ------------------------------------------------------------------- driver
_CACHE = {}


def run(inputs, n=50000, nfeat=512, nhid1=256, nhid2=64, att_h=16, trace=False):
    plan, in_maps = make_plan(n, nfeat, nhid1, nhid2, att_h, inputs)
    key = (n, nfeat, tuple(int(t) for tt in plan['T'] for t in tt))
    if key in _CACHE:
        nc = _CACHE[key]
    else:
        nc = build_program(plan)
        _CACHE[key] = nc
    res = bass_utils.run_bass_kernel_spmd(
        nc, in_maps, core_ids=list(range(NC)), trace=trace,
        trace_cores=list(range(NC)) if trace else None)
    nper = plan['nper']
    outs = []
    for name in ('emb1', 'emb2', 'emb', 'pi', 'var', 'mean'):
        full = np.concatenate(
            [res.results[c][name][:nper] for c in range(NC)], axis=0)
        outs.append(full)
    return tuple(outs), res


def kernel(**inputs):
    outs, _ = run(inputs)
    return outs
